# revision 3
# baseline (speedup 1.0000x reference)
"""MemN2N dialog kernel for 8 Trainium2 NeuronCores (SPMD).

Sharding: data-parallel over batch B=64 (8 per core) for the story/query
embedding sums and hops; candidate scoring sharded over C=10000 (1250 per
core). Embedding tables A and W are replicated in each core's DRAM and
gathered on-device via indirect (dynamic-offset) DMAs with fused CCE-add
accumulation — one instruction gathers 128 rows (one per SBUF partition) and
adds them into the per-cell accumulator, so the token-sum reduction happens
inside the DMA datapath. A 4KB AllGather shares the per-core hop output u
across cores for the final u @ cand.T scoring matmul.

Self-contained: hardcodes shapes from the problem spec
(B=64, M=200, S=50, C=10000, VOCAB=32000, E=64, HOPS=3).
"""

import sys

sys.path.insert(0, "/opt/trn_rl_repo")

import numpy as np

import concourse.bass as bass
import concourse.tile as tile
from concourse import bacc, mybir
from concourse.bass_utils import run_bass_kernel_spmd

NCORES = 8
VOCAB = 32000
E = 64          # embedding size; concat word+mask -> 2E = 128
TWO_E = 128
HOPS = 3
B, M, S, C = 64, 200, 50, 10000
BL = B // NCORES          # 8 batches per core
CL = C // NCORES          # 1250 candidates per core

# story/query cell layout (per core): cells are batch-major, cell = b*M + m
N_STORY = BL * M                     # 1600 story cells
N_WORD = N_STORY + 2 * BL            # + 8 query-word + 8 query-mask cells
N_TILES_S = 13                       # ceil(1616/128) -> 1664 slots
N_TILES_C = 10                       # ceil(1250/128) -> 1280 slots
CAND_SLOTS = N_TILES_C * 128         # 1280

_CACHE = {}


def _build_nc(use_collective=True):
    nc = bacc.Bacc("TRN2", target_bir_lowering=False, debug=False,
                   num_devices=NCORES)
    dt = mybir.dt
    emb_A = nc.dram_tensor("emb_A", [VOCAB, E], dt.float32, kind="ExternalInput").ap()
    emb_W = nc.dram_tensor("emb_W", [VOCAB, E], dt.float32, kind="ExternalInput").ap()
    # token indices per cell-tile: [tile, partition(cell), token]
    idx_s = nc.dram_tensor("idx_s", [2 * N_TILES_S, 128, S], dt.int32, kind="ExternalInput").ap()
    idx_c = nc.dram_tensor("idx_c", [2 * N_TILES_C, 128, S], dt.int32, kind="ExternalInput").ap()
    hwT = nc.dram_tensor("hwT", [TWO_E, TWO_E], dt.float32, kind="ExternalInput").ap()
    hb = nc.dram_tensor("hb", [TWO_E, 1], dt.float32, kind="ExternalInput").ap()
    ident = nc.dram_tensor("ident", [128, 128], dt.float32, kind="ExternalInput").ap()
    amask = nc.dram_tensor("amask", [BL, N_STORY], dt.float32, kind="ExternalInput").ap()
    logits_out = nc.dram_tensor("logits", [B, CAND_SLOTS], dt.float32, kind="ExternalOutput").ap()

    cc_in = nc.dram_tensor("cc_in", [TWO_E, BL], dt.float32)
    cc_out = nc.dram_tensor("cc_out", [NCORES, TWO_E, BL], dt.float32, addr_space="Shared")
    u_out = nc.dram_tensor("u_part", [TWO_E, BL], dt.float32, kind="ExternalOutput").ap()

    with tile.TileContext(nc) as tc:
        with (
            tc.tile_pool(name="idxp", bufs=4) as idxp,
            tc.tile_pool(name="mp", bufs=1) as mp,          # persistent m / cand tiles
            tc.tile_pool(name="mtp", bufs=1) as mtp,        # mT / candT
            tc.tile_pool(name="cons", bufs=1) as cons,      # constants
            tc.tile_pool(name="work", bufs=2) as work,
            tc.tile_pool(name="ps", bufs=1, space="PSUM") as ps,
            tc.tile_pool(name="ps_big", bufs=1, space="PSUM") as ps_big,
        ):
            ident_sb = cons.tile([128, 128], dt.float32)
            nc.sync.dma_start(out=ident_sb[:], in_=ident)
            hwT_sb = cons.tile([TWO_E, TWO_E], dt.float32)
            nc.sync.dma_start(out=hwT_sb[:], in_=hwT)
            hb_sb = cons.tile([TWO_E, 1], dt.float32)
            nc.sync.dma_start(out=hb_sb[:], in_=hb)
            amask_sb = cons.tile([BL, N_STORY], dt.float32)
            nc.sync.dma_start(out=amask_sb[:], in_=amask)

            def gather_sum(dst_ap, idx_dram_tile, table):
                """dst[p, :] = sum_s table[idx[p, s], :] via fused indirect adds."""
                idx_sb = idxp.tile([128, S], dt.int32)
                nc.sync.dma_start(out=idx_sb[:], in_=idx_dram_tile)
                for s in range(S):
                    nc.gpsimd.indirect_dma_start(
                        out=dst_ap,
                        out_offset=None,
                        in_=table,
                        in_offset=bass.IndirectOffsetOnAxis(ap=idx_sb[:, s:s + 1], axis=0),
                        compute_op=mybir.AluOpType.bypass if s == 0 else mybir.AluOpType.add,
                    )

            # ---- story memory m (and query u0) ----
            m_sb = [mp.tile([128, TWO_E], dt.float32, tag=f"m{t}", name=f"m{t}") for t in range(N_TILES_S)]
            for t in range(N_TILES_S):
                gather_sum(m_sb[t][:, 0:E], idx_s[t], emb_A)          # word half
                gather_sum(m_sb[t][:, E:TWO_E], idx_s[N_TILES_S + t], emb_A)  # mask half

            # mT [128e, 1664 cells]
            mT = mtp.tile([128, N_TILES_S * 128], dt.float32)
            for t in range(N_TILES_S):
                pt = ps.tile([128, 512], dt.float32, tag="pp512")
                nc.tensor.transpose(out=pt[:, 0:128], in_=m_sb[t][:], identity=ident_sb[:])
                nc.scalar.copy(mT[:, 128 * t:128 * (t + 1)], pt[:, 0:128])

            # u0^T [128, 8]: query cells live in tile 12, partitions 64..79
            qcat = work.tile([2 * BL, TWO_E], dt.float32, tag="qcat")
            nc.sync.dma_start(out=qcat[0:BL, 0:E], in_=m_sb[12][64:64 + BL, 0:E])
            nc.sync.dma_start(out=qcat[0:BL, E:TWO_E], in_=m_sb[12][64 + BL:64 + 2 * BL, 0:E])
            up = ps.tile([TWO_E, BL], dt.float32, tag="pu")
            nc.tensor.transpose(out=up[:], in_=qcat[0:BL, :], identity=ident_sb[0:BL, 0:BL])
            uT = work.tile([TWO_E, BL], dt.float32, tag="uT")
            nc.vector.tensor_copy(uT[:], up[:])

            # ---- candidates ----
            cand_sb = [mp.tile([128, TWO_E], dt.float32, tag=f"c{t}", name=f"c{t}") for t in range(N_TILES_C)]
            for t in range(N_TILES_C):
                gather_sum(cand_sb[t][:, 0:E], idx_c[t], emb_W)
                gather_sum(cand_sb[t][:, E:TWO_E], idx_c[N_TILES_C + t], emb_W)
            candT = mtp.tile([128, CAND_SLOTS], dt.float32)
            for t in range(N_TILES_C):
                pt = ps.tile([128, 512], dt.float32, tag="pp512")
                nc.tensor.transpose(out=pt[:, 0:128], in_=cand_sb[t][:], identity=ident_sb[:])
                nc.scalar.copy(candT[:, 128 * t:128 * (t + 1)], pt[:, 0:128])

            # ---- hops ----
            for h in range(HOPS):
                ap = ps_big.tile([BL, 2048], dt.float32, tag="attn")
                for j, (c0, c1) in enumerate([(0, 512), (512, 1024), (1024, 1536), (1536, 1600)]):
                    nc.tensor.matmul(out=ap[:, c0:c1], lhsT=uT[:], rhs=mT[:, c0:c1],
                                     start=True, stop=True)
                masked = work.tile([BL, N_STORY], dt.float32, tag="masked")
                nc.vector.tensor_tensor(out=masked[:], in0=ap[:, 0:N_STORY], in1=amask_sb[:],
                                        op=mybir.AluOpType.mult)
                nmax = work.tile([BL, 1], dt.float32, tag="nmax")
                nc.vector.tensor_reduce(out=nmax[:], in_=masked[:], axis=mybir.AxisListType.X,
                                        op=mybir.AluOpType.max, negate=True)
                esb = work.tile([BL, N_STORY], dt.float32, tag="esb")
                nc.scalar.activation(esb[:], masked[:], mybir.ActivationFunctionType.Exp,
                                     bias=nmax[:], scale=1.0)
                e2 = work.tile([BL, N_STORY], dt.float32, tag="e2")
                nc.vector.tensor_tensor(out=e2[:], in0=esb[:], in1=amask_sb[:],
                                        op=mybir.AluOpType.mult)
                ssum = work.tile([BL, 1], dt.float32, tag="ssum")
                nc.vector.tensor_reduce(out=ssum[:], in_=e2[:], axis=mybir.AxisListType.X,
                                        op=mybir.AluOpType.add)
                rinv = work.tile([BL, 1], dt.float32, tag="rinv")
                nc.vector.reciprocal(rinv[:], ssum[:])
                attn = work.tile([BL, N_STORY], dt.float32, tag="attn_sb")
                nc.vector.tensor_scalar_mul(attn[:], e2[:], rinv[:])

                # u_new^T = oT + H_w @ uT (+ H_b)
                pu = ps.tile([TWO_E, BL], dt.float32, tag="pu")
                for t in range(N_TILES_S):
                    k = 128 if t < 12 else 64  # tile 12: only 64 story cells
                    at = ps.tile([128, 512], dt.float32, tag="pp512")
                    nc.tensor.transpose(out=at[0:k, 0:BL], in_=attn[:, 128 * t:128 * t + k],
                                        identity=ident_sb[0:BL, 0:BL])
                    at_sb = work.tile([128, BL], dt.float32, tag="attnT_sb")
                    nc.vector.tensor_copy(at_sb[0:k, :], at[0:k, 0:BL])
                    nc.tensor.matmul(out=pu[:], lhsT=m_sb[t][0:k, :], rhs=at_sb[0:k, :],
                                     start=(t == 0), stop=False)
                nc.tensor.matmul(out=pu[:], lhsT=hwT_sb[:], rhs=uT[:], start=False, stop=True)
                uT = work.tile([TWO_E, BL], dt.float32, tag="uT")
                nc.scalar.activation(uT[:], pu[:], mybir.ActivationFunctionType.Identity,
                                     bias=hb_sb[:], scale=1.0)

            # ---- share u across cores ----
            nc.sync.dma_start(out=cc_in.ap(), in_=uT[:])
            if use_collective:
                nc.gpsimd.collective_compute(
                    "AllGather",
                    mybir.AluOpType.bypass,
                    replica_groups=[list(range(NCORES))],
                    ins=[cc_in.ap()],
                    outs=[cc_out.ap()],
                )
                uall = work.tile([TWO_E, NCORES, BL], dt.float32, tag="uall")
                # uall[p, r, b] = cc_out[r, p, b]
                nc.sync.dma_start(out=uall[:], in_=cc_out.ap().rearrange("r p b -> p r b"))

                lg = work.tile([B, CAND_SLOTS], dt.float32, tag="lg")
                for (c0, c1) in [(0, 512), (512, 1024), (1024, 1280)]:
                    pl = ps.tile([B, 512], dt.float32, tag="pp512")
                    nc.tensor.matmul(out=pl[:, 0:c1 - c0],
                                     lhsT=uall[:].rearrange("p r b -> p (r b)"),
                                     rhs=candT[:, c0:c1], start=True, stop=True)
                    nc.scalar.copy(lg[:, c0:c1], pl[:, 0:c1 - c0])
                nc.sync.dma_start(out=logits_out, in_=lg[:])
            else:
                # fallback: per-core partial logits for local batches vs local cands
                lg = work.tile([BL, CAND_SLOTS], dt.float32, tag="lgf")
                for (c0, c1) in [(0, 512), (512, 1024), (1024, 1280)]:
                    pl = ps.tile([BL, 512], dt.float32, tag="pp512")
                    nc.tensor.matmul(out=pl[:, 0:c1 - c0], lhsT=uT[:],
                                     rhs=candT[:, c0:c1], start=True, stop=True)
                    nc.scalar.copy(lg[:, c0:c1], pl[:, 0:c1 - c0])
                nc.sync.dma_start(out=logits_out[0:BL, :], in_=lg[:])
            nc.sync.dma_start(out=u_out, in_=uT[:])
    nc.compile()
    return nc


def _pad_to(a, n, fill=0):
    out = np.full((n,) + a.shape[1:], fill, a.dtype)
    out[: a.shape[0]] = a
    return out


def _build_in_maps(stories, query, stories_mask, query_mask, candidates,
                   candidates_mask, A, W, H_w, H_b):
    f32 = np.float32
    emb_A = np.ascontiguousarray(A, dtype=f32)
    emb_W = np.ascontiguousarray(W, dtype=f32)
    hwT = np.ascontiguousarray(H_w.T, dtype=f32)
    hb = np.ascontiguousarray(H_b, dtype=f32).reshape(TWO_E, 1)
    ident = np.eye(128, dtype=f32)
    # attention validity mask: batch b owns cells [b*M, (b+1)*M)
    amask = np.zeros((BL, N_STORY), f32)
    for b in range(BL):
        amask[b, b * M:(b + 1) * M] = 1.0

    in_maps = []
    for c in range(NCORES):
        bs = slice(c * BL, (c + 1) * BL)
        st = np.asarray(stories[bs], dtype=np.int32).reshape(N_STORY, S)   # cell = b*M+m
        stm = np.asarray(stories_mask[bs], dtype=np.int32).reshape(N_STORY, S)
        q = np.asarray(query[bs], dtype=np.int32)                          # [BL, S]
        qm = np.asarray(query_mask[bs], dtype=np.int32)

        word_cells = np.concatenate([st, q, qm], axis=0)                   # [1616, S]
        word_cells = _pad_to(word_cells, N_TILES_S * 128)
        mask_cells = _pad_to(stm, N_TILES_S * 128)
        idx_s = np.concatenate([word_cells, mask_cells], axis=0).reshape(
            2 * N_TILES_S, 128, S)

        cw = np.asarray(candidates[c * CL:(c + 1) * CL], dtype=np.int32)   # [1250, S]
        cm = np.asarray(candidates_mask[c * CL:(c + 1) * CL], dtype=np.int32)
        idx_c = np.concatenate([_pad_to(cw, CAND_SLOTS), _pad_to(cm, CAND_SLOTS)],
                               axis=0).reshape(2 * N_TILES_C, 128, S)

        in_maps.append({
            "emb_A": emb_A, "emb_W": emb_W,
            "idx_s": np.ascontiguousarray(idx_s),
            "idx_c": np.ascontiguousarray(idx_c),
            "hwT": hwT, "hb": hb, "ident": ident, "amask": amask,
        })
    return in_maps


def kernel(stories, query, stories_mask, query_mask, candidates,
           candidates_mask, A, W, H_w, H_b):
    use_collective = _CACHE.setdefault("use_collective", True)
    key = ("nc", use_collective)
    if key not in _CACHE:
        _CACHE[key] = _build_nc(use_collective)
    nc = _CACHE[key]
    in_maps = _build_in_maps(stories, query, stories_mask, query_mask,
                             candidates, candidates_mask, A, W, H_w, H_b)
    res = run_bass_kernel_spmd(nc, in_maps, list(range(NCORES))).results
    if use_collective:
        logits = np.concatenate([res[c]["logits"][:, :CL] for c in range(NCORES)], axis=1)
    else:
        u_full = np.concatenate([res[c]["u_part"].T for c in range(NCORES)], axis=0)  # [B, 2E]
        cand_cols = []  # not available in fallback without extra outputs
        raise NotImplementedError
    return logits.astype(np.float32)


if __name__ == "__main__":
    # quick self-run against reference when executed inside /root/problem
    sys.path.insert(0, "/root/problem")
    import reference
    inputs = {k: np.asarray(v) for k, v in reference.setup_inputs().items()}
    got = kernel(**inputs)
    exp = np.asarray(reference.reference(**reference.setup_inputs()))
    err = np.abs(got - exp).max() / (np.abs(exp).max() + 1e-9)
    print("rel err:", err)


# revision 4
# speedup vs baseline: 1.1116x; 1.1116x over previous
"""MemN2N dialog kernel for 8 Trainium2 NeuronCores (SPMD).

Sharding: data-parallel over batch B=64 (8 per core) for the story/query
embedding sums and hops; candidate scoring sharded over C=10000 (1250 per
core). Embedding tables A and W are replicated in each core's DRAM and
gathered on-device via indirect (dynamic-offset) DMAs with fused CCE-add
accumulation — one instruction gathers 128 rows (one per SBUF partition) and
adds them into the per-cell accumulator, so the token-sum reduction happens
inside the DMA datapath. A 4KB AllGather shares the per-core hop output u
across cores for the final u @ cand.T scoring matmul.

Self-contained: hardcodes shapes from the problem spec
(B=64, M=200, S=50, C=10000, VOCAB=32000, E=64, HOPS=3).
"""

import sys

sys.path.insert(0, "/opt/trn_rl_repo")

import numpy as np

import concourse.bass as bass
import concourse.tile as tile
from concourse import bacc, mybir
from concourse.bass_utils import run_bass_kernel_spmd

NCORES = 8
VOCAB = 32000
E = 64          # embedding size; concat word+mask -> 2E = 128
TWO_E = 128
HOPS = 3
B, M, S, C = 64, 200, 50, 10000
BL = B // NCORES          # 8 batches per core
CL = C // NCORES          # 1250 candidates per core

# story/query cell layout (per core): cells are batch-major, cell = b*M + m
N_STORY = BL * M                     # 1600 story cells
N_WORD = N_STORY + 2 * BL            # + 8 query-word + 8 query-mask cells
N_TILES_S = 13                       # ceil(1616/128) -> 1664 slots
N_TILES_C = 10                       # ceil(1250/128) -> 1280 slots
CAND_SLOTS = N_TILES_C * 128         # 1280

_CACHE = {}


def _build_nc(use_collective=True):
    nc = bacc.Bacc("TRN2", target_bir_lowering=False, debug=False,
                   num_devices=NCORES)
    dt = mybir.dt
    emb_A = nc.dram_tensor("emb_A", [VOCAB, E], dt.float32, kind="ExternalInput").ap()
    emb_W = nc.dram_tensor("emb_W", [VOCAB, E], dt.float32, kind="ExternalInput").ap()
    # token indices per cell-tile: [tile, partition(cell), token]
    idx_s = nc.dram_tensor("idx_s", [2 * N_TILES_S, 128, S], dt.int32, kind="ExternalInput").ap()
    idx_c = nc.dram_tensor("idx_c", [2 * N_TILES_C, 128, S], dt.int32, kind="ExternalInput").ap()
    hwT = nc.dram_tensor("hwT", [TWO_E, TWO_E], dt.float32, kind="ExternalInput").ap()
    hb = nc.dram_tensor("hb", [TWO_E, 1], dt.float32, kind="ExternalInput").ap()
    ident = nc.dram_tensor("ident", [128, 128], dt.float32, kind="ExternalInput").ap()
    amask = nc.dram_tensor("amask", [BL, N_STORY], dt.float32, kind="ExternalInput").ap()
    logits_out = nc.dram_tensor("logits", [B, CAND_SLOTS], dt.float32, kind="ExternalOutput").ap()

    cc_in = nc.dram_tensor("cc_in", [TWO_E, BL], dt.float32)
    cc_out = nc.dram_tensor("cc_out", [NCORES, TWO_E, BL], dt.float32, addr_space="Shared")
    u_out = nc.dram_tensor("u_part", [TWO_E, BL], dt.float32, kind="ExternalOutput").ap()

    with tile.TileContext(nc) as tc:
        with (
            tc.tile_pool(name="idxp", bufs=4) as idxp,
            tc.tile_pool(name="mp", bufs=1) as mp,          # persistent m / cand tiles
            tc.tile_pool(name="mtp", bufs=1) as mtp,        # mT / candT
            tc.tile_pool(name="cons", bufs=1) as cons,      # constants
            tc.tile_pool(name="work", bufs=2) as work,
            tc.tile_pool(name="ps", bufs=1, space="PSUM") as ps,
            tc.tile_pool(name="ps_big", bufs=1, space="PSUM") as ps_big,
        ):
            ident_sb = cons.tile([128, 128], dt.float32)
            nc.sync.dma_start(out=ident_sb[:], in_=ident)
            hwT_sb = cons.tile([TWO_E, TWO_E], dt.float32)
            nc.sync.dma_start(out=hwT_sb[:], in_=hwT)
            hb_sb = cons.tile([TWO_E, 1], dt.float32)
            nc.sync.dma_start(out=hb_sb[:], in_=hb)
            amask_sb = cons.tile([BL, N_STORY], dt.float32)
            nc.sync.dma_start(out=amask_sb[:], in_=amask)

            def gather_sum(dst_ap, idx_dram_tile, table):
                """dst[p, :] = sum_s table[idx[p, s], :] via fused indirect adds."""
                idx_sb = idxp.tile([128, S], dt.int32)
                nc.sync.dma_start(out=idx_sb[:], in_=idx_dram_tile)
                for s in range(S):
                    nc.gpsimd.indirect_dma_start(
                        out=dst_ap,
                        out_offset=None,
                        in_=table,
                        in_offset=bass.IndirectOffsetOnAxis(ap=idx_sb[:, s:s + 1], axis=0),
                        compute_op=mybir.AluOpType.bypass if s == 0 else mybir.AluOpType.add,
                    )

            # ---- story memory m (and query u0) ----
            m_sb = [mp.tile([128, TWO_E], dt.float32, tag=f"m{t}", name=f"m{t}") for t in range(N_TILES_S)]
            for t in range(N_TILES_S):
                gather_sum(m_sb[t][:, 0:E], idx_s[t], emb_A)          # word half
                gather_sum(m_sb[t][:, E:TWO_E], idx_s[N_TILES_S + t], emb_A)  # mask half

            # mT [128e, 1664 cells]
            mT = mtp.tile([128, N_TILES_S * 128], dt.float32)
            for t in range(N_TILES_S):
                pt = ps.tile([128, 512], dt.float32, tag="pp512")
                nc.tensor.transpose(out=pt[:, 0:128], in_=m_sb[t][:], identity=ident_sb[:])
                nc.scalar.copy(mT[:, 128 * t:128 * (t + 1)], pt[:, 0:128])

            # u0^T [128, 8]: query cells live in tile 12, partitions 64..79
            qcat = work.tile([2 * BL, TWO_E], dt.float32, tag="qcat")
            nc.sync.dma_start(out=qcat[0:BL, 0:E], in_=m_sb[12][64:64 + BL, 0:E])
            nc.sync.dma_start(out=qcat[0:BL, E:TWO_E], in_=m_sb[12][64 + BL:64 + 2 * BL, 0:E])
            up = ps.tile([TWO_E, BL], dt.float32, tag="pu")
            nc.tensor.transpose(out=up[:], in_=qcat[0:BL, :], identity=ident_sb[0:BL, 0:BL])
            uT = work.tile([TWO_E, BL], dt.float32, tag="uT")
            nc.vector.tensor_copy(uT[:], up[:])

            # ---- candidates ----
            cand_sb = [mp.tile([128, TWO_E], dt.float32, tag=f"c{t}", name=f"c{t}") for t in range(N_TILES_C)]
            for t in range(N_TILES_C):
                gather_sum(cand_sb[t][:, 0:E], idx_c[t], emb_W)
                gather_sum(cand_sb[t][:, E:TWO_E], idx_c[N_TILES_C + t], emb_W)
            candT = mtp.tile([128, CAND_SLOTS], dt.float32)
            for t in range(N_TILES_C):
                pt = ps.tile([128, 512], dt.float32, tag="pp512")
                nc.tensor.transpose(out=pt[:, 0:128], in_=cand_sb[t][:], identity=ident_sb[:])
                nc.scalar.copy(candT[:, 128 * t:128 * (t + 1)], pt[:, 0:128])

            # ---- hops ----
            for h in range(HOPS):
                ap = ps_big.tile([BL, 2048], dt.float32, tag="attn")
                for j, (c0, c1) in enumerate([(0, 512), (512, 1024), (1024, 1536), (1536, 1600)]):
                    nc.tensor.matmul(out=ap[:, c0:c1], lhsT=uT[:], rhs=mT[:, c0:c1],
                                     start=True, stop=True)
                masked = work.tile([BL, N_STORY], dt.float32, tag="masked")
                nc.vector.tensor_tensor(out=masked[:], in0=ap[:, 0:N_STORY], in1=amask_sb[:],
                                        op=mybir.AluOpType.mult)
                nmax = work.tile([BL, 1], dt.float32, tag="nmax")
                nc.vector.tensor_reduce(out=nmax[:], in_=masked[:], axis=mybir.AxisListType.X,
                                        op=mybir.AluOpType.max, negate=True)
                esb = work.tile([BL, N_STORY], dt.float32, tag="esb")
                nc.scalar.activation(esb[:], masked[:], mybir.ActivationFunctionType.Exp,
                                     bias=nmax[:], scale=1.0)
                e2 = work.tile([BL, N_STORY], dt.float32, tag="e2")
                nc.vector.tensor_tensor(out=e2[:], in0=esb[:], in1=amask_sb[:],
                                        op=mybir.AluOpType.mult)
                ssum = work.tile([BL, 1], dt.float32, tag="ssum")
                nc.vector.tensor_reduce(out=ssum[:], in_=e2[:], axis=mybir.AxisListType.X,
                                        op=mybir.AluOpType.add)
                rinv = work.tile([BL, 1], dt.float32, tag="rinv")
                nc.vector.reciprocal(rinv[:], ssum[:])
                attn = work.tile([BL, N_STORY], dt.float32, tag="attn_sb")
                nc.vector.tensor_scalar_mul(attn[:], e2[:], rinv[:])

                # u_new^T = oT + H_w @ uT (+ H_b)
                pu = ps.tile([TWO_E, BL], dt.float32, tag="pu")
                for t in range(N_TILES_S):
                    k = 128 if t < 12 else 64  # tile 12: only 64 story cells
                    at = ps.tile([128, 512], dt.float32, tag="pp512")
                    nc.tensor.transpose(out=at[0:k, 0:BL], in_=attn[:, 128 * t:128 * t + k],
                                        identity=ident_sb[0:BL, 0:BL])
                    at_sb = work.tile([128, BL], dt.float32, tag="attnT_sb")
                    nc.vector.tensor_copy(at_sb[0:k, :], at[0:k, 0:BL])
                    nc.tensor.matmul(out=pu[:], lhsT=m_sb[t][0:k, :], rhs=at_sb[0:k, :],
                                     start=(t == 0), stop=False)
                nc.tensor.matmul(out=pu[:], lhsT=hwT_sb[:], rhs=uT[:], start=False, stop=True)
                uT = work.tile([TWO_E, BL], dt.float32, tag="uT")
                nc.scalar.activation(uT[:], pu[:], mybir.ActivationFunctionType.Identity,
                                     bias=hb_sb[:], scale=1.0)

            # ---- share u across cores ----
            nc.sync.dma_start(out=cc_in.ap(), in_=uT[:])
            if use_collective:
                nc.gpsimd.collective_compute(
                    "AllGather",
                    mybir.AluOpType.bypass,
                    replica_groups=[list(range(NCORES))],
                    ins=[cc_in.ap()],
                    outs=[cc_out.ap()],
                )
                uall = work.tile([TWO_E, NCORES, BL], dt.float32, tag="uall")
                # uall[p, r, b] = cc_out[r, p, b]
                nc.sync.dma_start(out=uall[:], in_=cc_out.ap().rearrange("r p b -> p r b"))

                lg = work.tile([B, CAND_SLOTS], dt.float32, tag="lg")
                for (c0, c1) in [(0, 512), (512, 1024), (1024, 1280)]:
                    pl = ps.tile([B, 512], dt.float32, tag="pp512")
                    nc.tensor.matmul(out=pl[:, 0:c1 - c0],
                                     lhsT=uall[:].rearrange("p r b -> p (r b)"),
                                     rhs=candT[:, c0:c1], start=True, stop=True)
                    nc.scalar.copy(lg[:, c0:c1], pl[:, 0:c1 - c0])
                nc.sync.dma_start(out=logits_out, in_=lg[:])
            else:
                # fallback: per-core partial logits for local batches vs local cands
                lg = work.tile([BL, CAND_SLOTS], dt.float32, tag="lgf")
                for (c0, c1) in [(0, 512), (512, 1024), (1024, 1280)]:
                    pl = ps.tile([BL, 512], dt.float32, tag="pp512")
                    nc.tensor.matmul(out=pl[:, 0:c1 - c0], lhsT=uT[:],
                                     rhs=candT[:, c0:c1], start=True, stop=True)
                    nc.scalar.copy(lg[:, c0:c1], pl[:, 0:c1 - c0])
                nc.sync.dma_start(out=logits_out[0:BL, :], in_=lg[:])
            nc.sync.dma_start(out=u_out, in_=uT[:])
    nc.compile()
    return nc


def _pad_to(a, n, fill=0):
    out = np.full((n,) + a.shape[1:], fill, a.dtype)
    out[: a.shape[0]] = a
    return out


def _build_in_maps(stories, query, stories_mask, query_mask, candidates,
                   candidates_mask, A, W, H_w, H_b):
    f32 = np.float32
    emb_A = np.ascontiguousarray(A, dtype=f32)
    emb_W = np.ascontiguousarray(W, dtype=f32)
    hwT = np.ascontiguousarray(H_w.T, dtype=f32)
    hb = np.ascontiguousarray(H_b, dtype=f32).reshape(TWO_E, 1)
    ident = np.eye(128, dtype=f32)
    # attention validity mask: batch b owns cells [b*M, (b+1)*M)
    amask = np.zeros((BL, N_STORY), f32)
    for b in range(BL):
        amask[b, b * M:(b + 1) * M] = 1.0

    in_maps = []
    for c in range(NCORES):
        bs = slice(c * BL, (c + 1) * BL)
        st = np.asarray(stories[bs], dtype=np.int32).reshape(N_STORY, S)   # cell = b*M+m
        stm = np.asarray(stories_mask[bs], dtype=np.int32).reshape(N_STORY, S)
        q = np.asarray(query[bs], dtype=np.int32)                          # [BL, S]
        qm = np.asarray(query_mask[bs], dtype=np.int32)

        word_cells = np.concatenate([st, q, qm], axis=0)                   # [1616, S]
        word_cells = _pad_to(word_cells, N_TILES_S * 128)
        mask_cells = _pad_to(stm, N_TILES_S * 128)
        idx_s = np.concatenate([word_cells, mask_cells], axis=0).reshape(
            2 * N_TILES_S, 128, S)

        cw = np.asarray(candidates[c * CL:(c + 1) * CL], dtype=np.int32)   # [1250, S]
        cm = np.asarray(candidates_mask[c * CL:(c + 1) * CL], dtype=np.int32)
        idx_c = np.concatenate([_pad_to(cw, CAND_SLOTS), _pad_to(cm, CAND_SLOTS)],
                               axis=0).reshape(2 * N_TILES_C, 128, S)

        in_maps.append({
            "emb_A": emb_A, "emb_W": emb_W,
            "idx_s": np.ascontiguousarray(idx_s),
            "idx_c": np.ascontiguousarray(idx_c),
            "hwT": hwT, "hb": hb, "ident": ident, "amask": amask,
        })
    return in_maps


def kernel(stories, query, stories_mask, query_mask, candidates,
           candidates_mask, A, W, H_w, H_b):
    if "nc" not in _CACHE:
        _CACHE["nc"] = _build_nc(use_collective=True)
    nc = _CACHE["nc"]
    in_maps = _build_in_maps(stories, query, stories_mask, query_mask,
                             candidates, candidates_mask, A, W, H_w, H_b)
    res = run_bass_kernel_spmd(nc, in_maps, list(range(NCORES))).results
    # core c computed logits for ALL 64 batches against its 1250-candidate shard
    logits = np.concatenate([res[c]["logits"][:, :CL] for c in range(NCORES)], axis=1)
    return logits.astype(np.float32)


if __name__ == "__main__":
    # quick self-run against reference when executed inside /root/problem
    sys.path.insert(0, "/root/problem")
    import reference
    inputs = {k: np.asarray(v) for k, v in reference.setup_inputs().items()}
    got = kernel(**inputs)
    exp = np.asarray(reference.reference(**reference.setup_inputs()))
    err = np.abs(got - exp).max() / (np.abs(exp).max() + 1e-9)
    print("rel err:", err)


# revision 5
# speedup vs baseline: 1.6287x; 1.4652x over previous
"""MemN2N dialog kernel for 8 Trainium2 NeuronCores (SPMD).

Sharding: data-parallel over batch B=64 (8 per core) for the story/query
embedding sums and hops; candidate scoring sharded over C=10000 (1250 per
core). Embedding tables A and W are replicated in each core's DRAM and
gathered on-device via indirect (dynamic-offset) DMAs with fused CCE-add
accumulation — one instruction gathers 128 rows (one per SBUF partition) and
adds them into the per-cell accumulator, so the token-sum reduction happens
inside the DMA datapath. A 4KB AllGather shares the per-core hop output u
across cores for the final u @ cand.T scoring matmul.

Self-contained: hardcodes shapes from the problem spec
(B=64, M=200, S=50, C=10000, VOCAB=32000, E=64, HOPS=3).
"""

import sys

sys.path.insert(0, "/opt/trn_rl_repo")

import numpy as np

import concourse.bass as bass
import concourse.tile as tile
from concourse import bacc, mybir
from concourse.bass_utils import run_bass_kernel_spmd

NCORES = 8
VOCAB = 32000
E = 64          # embedding size; concat word+mask -> 2E = 128
TWO_E = 128
HOPS = 3
B, M, S, C = 64, 200, 50, 10000
BL = B // NCORES          # 8 batches per core
CL = C // NCORES          # 1250 candidates per core

# story/query cell layout (per core): cells are batch-major, cell = b*M + m
N_STORY = BL * M                     # 1600 story cells
N_WORD = N_STORY + 2 * BL            # + 8 query-word + 8 query-mask cells
N_TILES_S = 13                       # ceil(1616/128) -> 1664 slots
N_TILES_C = 10                       # ceil(1250/128) -> 1280 slots
CAND_SLOTS = N_TILES_C * 128         # 1280

_CACHE = {}


def _build_nc(use_collective=True):
    nc = bacc.Bacc("TRN2", target_bir_lowering=False, debug=False,
                   num_devices=NCORES)
    dt = mybir.dt
    emb_A = nc.dram_tensor("emb_A", [VOCAB, E], dt.bfloat16, kind="ExternalInput").ap()
    emb_W = nc.dram_tensor("emb_W", [VOCAB, E], dt.bfloat16, kind="ExternalInput").ap()
    # token indices per cell-tile: [tile, partition(cell), token]
    idx_s = nc.dram_tensor("idx_s", [2 * N_TILES_S, 128, S], dt.int32, kind="ExternalInput").ap()
    idx_c = nc.dram_tensor("idx_c", [2 * N_TILES_C, 128, S], dt.int32, kind="ExternalInput").ap()
    hwT = nc.dram_tensor("hwT", [TWO_E, TWO_E], dt.float32, kind="ExternalInput").ap()
    hb = nc.dram_tensor("hb", [TWO_E, 1], dt.float32, kind="ExternalInput").ap()
    ident = nc.dram_tensor("ident", [128, 128], dt.float32, kind="ExternalInput").ap()
    amask = nc.dram_tensor("amask", [BL, N_STORY], dt.float32, kind="ExternalInput").ap()
    logits_out = nc.dram_tensor("logits", [B, CAND_SLOTS], dt.float32, kind="ExternalOutput").ap()

    cc_in = nc.dram_tensor("cc_in", [TWO_E, BL], dt.float32)
    cc_out = nc.dram_tensor("cc_out", [NCORES, TWO_E, BL], dt.float32, addr_space="Shared")
    u_out = nc.dram_tensor("u_part", [TWO_E, BL], dt.float32, kind="ExternalOutput").ap()

    with tile.TileContext(nc) as tc:
        with (
            tc.tile_pool(name="idxp", bufs=4) as idxp,
            tc.tile_pool(name="mp", bufs=1) as mp,          # persistent m / cand tiles
            tc.tile_pool(name="mtp", bufs=1) as mtp,        # mT / candT
            tc.tile_pool(name="cons", bufs=1) as cons,      # constants
            tc.tile_pool(name="work", bufs=2) as work,
            tc.tile_pool(name="ps", bufs=1, space="PSUM") as ps,
            tc.tile_pool(name="ps_big", bufs=1, space="PSUM") as ps_big,
        ):
            ident_sb = cons.tile([128, 128], dt.float32)
            nc.sync.dma_start(out=ident_sb[:], in_=ident)
            hwT_sb = cons.tile([TWO_E, TWO_E], dt.float32)
            nc.sync.dma_start(out=hwT_sb[:], in_=hwT)
            hb_sb = cons.tile([TWO_E, 1], dt.float32)
            nc.sync.dma_start(out=hb_sb[:], in_=hb)
            amask_sb = cons.tile([BL, N_STORY], dt.float32)
            nc.sync.dma_start(out=amask_sb[:], in_=amask)

            def gather_sum(dst_ap, idx_dram_tile, table):
                """dst[p, :] = sum_s table[idx[p, s], :] via fused indirect adds."""
                idx_sb = idxp.tile([128, S], dt.int32)
                nc.sync.dma_start(out=idx_sb[:], in_=idx_dram_tile)
                for s in range(S):
                    nc.gpsimd.indirect_dma_start(
                        out=dst_ap,
                        out_offset=None,
                        in_=table,
                        in_offset=bass.IndirectOffsetOnAxis(ap=idx_sb[:, s:s + 1], axis=0),
                        compute_op=mybir.AluOpType.bypass if s == 0 else mybir.AluOpType.add,
                    )

            # ---- story memory m (and query u0) ----
            m_sb = [mp.tile([128, TWO_E], dt.float32, tag=f"m{t}", name=f"m{t}") for t in range(N_TILES_S)]
            for t in range(N_TILES_S):
                gather_sum(m_sb[t][:, 0:E], idx_s[t], emb_A)          # word half
                gather_sum(m_sb[t][:, E:TWO_E], idx_s[N_TILES_S + t], emb_A)  # mask half

            # mT [128e, 1664 cells]
            mT = mtp.tile([128, N_TILES_S * 128], dt.float32)
            for t in range(N_TILES_S):
                pt = ps.tile([128, 512], dt.float32, tag="pp512")
                nc.tensor.transpose(out=pt[:, 0:128], in_=m_sb[t][:], identity=ident_sb[:])
                nc.scalar.copy(mT[:, 128 * t:128 * (t + 1)], pt[:, 0:128])

            # u0^T [128, 8]: query cells live in tile 12, partitions 64..79
            qcat = work.tile([2 * BL, TWO_E], dt.float32, tag="qcat")
            nc.sync.dma_start(out=qcat[0:BL, 0:E], in_=m_sb[12][64:64 + BL, 0:E])
            nc.sync.dma_start(out=qcat[0:BL, E:TWO_E], in_=m_sb[12][64 + BL:64 + 2 * BL, 0:E])
            up = ps.tile([TWO_E, BL], dt.float32, tag="pu")
            nc.tensor.transpose(out=up[:], in_=qcat[0:BL, :], identity=ident_sb[0:BL, 0:BL])
            uT = work.tile([TWO_E, BL], dt.float32, tag="uT")
            nc.vector.tensor_copy(uT[:], up[:])

            # ---- candidates ----
            cand_sb = [mp.tile([128, TWO_E], dt.float32, tag=f"c{t}", name=f"c{t}") for t in range(N_TILES_C)]
            for t in range(N_TILES_C):
                gather_sum(cand_sb[t][:, 0:E], idx_c[t], emb_W)
                gather_sum(cand_sb[t][:, E:TWO_E], idx_c[N_TILES_C + t], emb_W)
            candT = mtp.tile([128, CAND_SLOTS], dt.float32)
            for t in range(N_TILES_C):
                pt = ps.tile([128, 512], dt.float32, tag="pp512")
                nc.tensor.transpose(out=pt[:, 0:128], in_=cand_sb[t][:], identity=ident_sb[:])
                nc.scalar.copy(candT[:, 128 * t:128 * (t + 1)], pt[:, 0:128])

            # ---- hops ----
            for h in range(HOPS):
                ap = ps_big.tile([BL, 2048], dt.float32, tag="attn")
                for j, (c0, c1) in enumerate([(0, 512), (512, 1024), (1024, 1536), (1536, 1600)]):
                    nc.tensor.matmul(out=ap[:, c0:c1], lhsT=uT[:], rhs=mT[:, c0:c1],
                                     start=True, stop=True)
                masked = work.tile([BL, N_STORY], dt.float32, tag="masked")
                nc.vector.tensor_tensor(out=masked[:], in0=ap[:, 0:N_STORY], in1=amask_sb[:],
                                        op=mybir.AluOpType.mult)
                nmax = work.tile([BL, 1], dt.float32, tag="nmax")
                nc.vector.tensor_reduce(out=nmax[:], in_=masked[:], axis=mybir.AxisListType.X,
                                        op=mybir.AluOpType.max, negate=True)
                esb = work.tile([BL, N_STORY], dt.float32, tag="esb")
                nc.scalar.activation(esb[:], masked[:], mybir.ActivationFunctionType.Exp,
                                     bias=nmax[:], scale=1.0)
                e2 = work.tile([BL, N_STORY], dt.float32, tag="e2")
                nc.vector.tensor_tensor(out=e2[:], in0=esb[:], in1=amask_sb[:],
                                        op=mybir.AluOpType.mult)
                ssum = work.tile([BL, 1], dt.float32, tag="ssum")
                nc.vector.tensor_reduce(out=ssum[:], in_=e2[:], axis=mybir.AxisListType.X,
                                        op=mybir.AluOpType.add)
                rinv = work.tile([BL, 1], dt.float32, tag="rinv")
                nc.vector.reciprocal(rinv[:], ssum[:])
                attn = work.tile([BL, N_STORY], dt.float32, tag="attn_sb")
                nc.vector.tensor_scalar_mul(attn[:], e2[:], rinv[:])

                # u_new^T = oT + H_w @ uT (+ H_b)
                pu = ps.tile([TWO_E, BL], dt.float32, tag="pu")
                for t in range(N_TILES_S):
                    k = 128 if t < 12 else 64  # tile 12: only 64 story cells
                    at = ps.tile([128, 512], dt.float32, tag="pp512")
                    nc.tensor.transpose(out=at[0:k, 0:BL], in_=attn[:, 128 * t:128 * t + k],
                                        identity=ident_sb[0:BL, 0:BL])
                    at_sb = work.tile([128, BL], dt.float32, tag="attnT_sb")
                    nc.vector.tensor_copy(at_sb[0:k, :], at[0:k, 0:BL])
                    nc.tensor.matmul(out=pu[:], lhsT=m_sb[t][0:k, :], rhs=at_sb[0:k, :],
                                     start=(t == 0), stop=False)
                nc.tensor.matmul(out=pu[:], lhsT=hwT_sb[:], rhs=uT[:], start=False, stop=True)
                uT = work.tile([TWO_E, BL], dt.float32, tag="uT")
                nc.scalar.activation(uT[:], pu[:], mybir.ActivationFunctionType.Identity,
                                     bias=hb_sb[:], scale=1.0)

            # ---- share u across cores ----
            nc.sync.dma_start(out=cc_in.ap(), in_=uT[:])
            if use_collective:
                nc.gpsimd.collective_compute(
                    "AllGather",
                    mybir.AluOpType.bypass,
                    replica_groups=[list(range(NCORES))],
                    ins=[cc_in.ap()],
                    outs=[cc_out.ap()],
                )
                uall = work.tile([TWO_E, NCORES, BL], dt.float32, tag="uall")
                # uall[p, r, b] = cc_out[r, p, b]
                nc.sync.dma_start(out=uall[:], in_=cc_out.ap().rearrange("r p b -> p r b"))

                lg = work.tile([B, CAND_SLOTS], dt.float32, tag="lg")
                for (c0, c1) in [(0, 512), (512, 1024), (1024, 1280)]:
                    pl = ps.tile([B, 512], dt.float32, tag="pp512")
                    nc.tensor.matmul(out=pl[:, 0:c1 - c0],
                                     lhsT=uall[:].rearrange("p r b -> p (r b)"),
                                     rhs=candT[:, c0:c1], start=True, stop=True)
                    nc.scalar.copy(lg[:, c0:c1], pl[:, 0:c1 - c0])
                nc.sync.dma_start(out=logits_out, in_=lg[:])
            else:
                # fallback: per-core partial logits for local batches vs local cands
                lg = work.tile([BL, CAND_SLOTS], dt.float32, tag="lgf")
                for (c0, c1) in [(0, 512), (512, 1024), (1024, 1280)]:
                    pl = ps.tile([BL, 512], dt.float32, tag="pp512")
                    nc.tensor.matmul(out=pl[:, 0:c1 - c0], lhsT=uT[:],
                                     rhs=candT[:, c0:c1], start=True, stop=True)
                    nc.scalar.copy(lg[:, c0:c1], pl[:, 0:c1 - c0])
                nc.sync.dma_start(out=logits_out[0:BL, :], in_=lg[:])
            nc.sync.dma_start(out=u_out, in_=uT[:])
    nc.compile()
    return nc


def _pad_to(a, n, fill=0):
    out = np.full((n,) + a.shape[1:], fill, a.dtype)
    out[: a.shape[0]] = a
    return out


def _build_in_maps(stories, query, stories_mask, query_mask, candidates,
                   candidates_mask, A, W, H_w, H_b):
    import ml_dtypes
    f32 = np.float32
    emb_A = np.ascontiguousarray(A, dtype=np.float32).astype(ml_dtypes.bfloat16)
    emb_W = np.ascontiguousarray(W, dtype=np.float32).astype(ml_dtypes.bfloat16)
    hwT = np.ascontiguousarray(H_w.T, dtype=f32)
    hb = np.ascontiguousarray(H_b, dtype=f32).reshape(TWO_E, 1)
    ident = np.eye(128, dtype=f32)
    # attention validity mask: batch b owns cells [b*M, (b+1)*M)
    amask = np.zeros((BL, N_STORY), f32)
    for b in range(BL):
        amask[b, b * M:(b + 1) * M] = 1.0

    in_maps = []
    for c in range(NCORES):
        bs = slice(c * BL, (c + 1) * BL)
        st = np.asarray(stories[bs], dtype=np.int32).reshape(N_STORY, S)   # cell = b*M+m
        stm = np.asarray(stories_mask[bs], dtype=np.int32).reshape(N_STORY, S)
        q = np.asarray(query[bs], dtype=np.int32)                          # [BL, S]
        qm = np.asarray(query_mask[bs], dtype=np.int32)

        word_cells = np.concatenate([st, q, qm], axis=0)                   # [1616, S]
        word_cells = _pad_to(word_cells, N_TILES_S * 128)
        mask_cells = _pad_to(stm, N_TILES_S * 128)
        idx_s = np.concatenate([word_cells, mask_cells], axis=0).reshape(
            2 * N_TILES_S, 128, S)

        cw = np.asarray(candidates[c * CL:(c + 1) * CL], dtype=np.int32)   # [1250, S]
        cm = np.asarray(candidates_mask[c * CL:(c + 1) * CL], dtype=np.int32)
        idx_c = np.concatenate([_pad_to(cw, CAND_SLOTS), _pad_to(cm, CAND_SLOTS)],
                               axis=0).reshape(2 * N_TILES_C, 128, S)

        in_maps.append({
            "emb_A": emb_A, "emb_W": emb_W,
            "idx_s": np.ascontiguousarray(idx_s),
            "idx_c": np.ascontiguousarray(idx_c),
            "hwT": hwT, "hb": hb, "ident": ident, "amask": amask,
        })
    return in_maps


def kernel(stories, query, stories_mask, query_mask, candidates,
           candidates_mask, A, W, H_w, H_b):
    if "nc" not in _CACHE:
        _CACHE["nc"] = _build_nc(use_collective=True)
    nc = _CACHE["nc"]
    in_maps = _build_in_maps(stories, query, stories_mask, query_mask,
                             candidates, candidates_mask, A, W, H_w, H_b)
    res = run_bass_kernel_spmd(nc, in_maps, list(range(NCORES))).results
    # core c computed logits for ALL 64 batches against its 1250-candidate shard
    logits = np.concatenate([res[c]["logits"][:, :CL] for c in range(NCORES)], axis=1)
    return logits.astype(np.float32)


if __name__ == "__main__":
    # quick self-run against reference when executed inside /root/problem
    sys.path.insert(0, "/root/problem")
    import reference
    inputs = {k: np.asarray(v) for k, v in reference.setup_inputs().items()}
    got = kernel(**inputs)
    exp = np.asarray(reference.reference(**reference.setup_inputs()))
    err = np.abs(got - exp).max() / (np.abs(exp).max() + 1e-9)
    print("rel err:", err)
